# revision 55
# baseline (speedup 1.0000x reference)
"""Multi-head attention (B=4, S=2048, E=1024, H=16, D=64) on 8 trn2 cores.

Sharding: core c handles batch b=c//2 and head-group hg=c%2 (8 heads, 512
embed cols). QKV projection weights are column-sharded by head group so
attention is fully local per device.

Per-core plan (bf16 matmul operands, fp32 PSUM accumulation):
  - X and W are cast to bf16 on the host; XT[e,q] is loaded straight from
    DRAM via xbar transpose-DMA (no PE transposes, no staging copies).
  - Wq/bq are pre-scaled by 16/ln2 on the host so the PSUM scores are
    directly the Schraudolph exponent; the ACT exp path compensates with
    scale=ln2/128.
  - QT[d,q], KT[d,q] = W.T @ XT (+bias via ACT Identity); V[s,d] (+bias via
    a K=128 unit-row matmul so the PE never leaves 128x128 mode), stored
    augmented [V | 1] per head.
  - scores^T[k,q] = KT.T @ QT per head-pair: two K=64 matmuls row-tiled at
    partition bases 0/64 (concurrent in the PE array). K-tiles are batched
    in PAIRS so the 64-row <-> 128-row tiling-mode drain is paid once per
    two k-tiles instead of twice per one.
  - e = exp(scores): even k-tiles on ACT (exact exp, scale=ln2/128), odd
    k-tiles on DVE via the Schraudolph bit-trick (round(sps+16248) as int16
    reinterpreted as bf16) to split the exp throughput across two engines.
  - ctxT_aug[65,q] += [V|1].T @ e ; row 64 = Z (softmax denominator).
  - output: DVE 32x32 block-transpose + per-q 1/Z scale (Z columns via a
    tiny DRAM bounce) + block-permuted DMA to DRAM. PSUM->SBUF ctx drains
    on ACT.
  - Prologue: XT transpose-DMAs on the sync queue (q-chunk major), W/bias
    loads on the gpsimd queue in parallel; V and Q/K chunk projections are
    injected just-in-time inside pair 0's first q-chunk.

Baseline 396us/core -> this version targets ~290us.
"""

import os
import numpy as np
from contextlib import ExitStack

import concourse.bass as bass
import concourse.mybir as mybir
import concourse.tile as tile
from concourse.bass import ts, ds

B, S, E = 4, 2048, 1024
H, DH = 16, 64
NCORES = 8
HG = 2                # head groups per batch (cores per batch element)
HPC = H // HG         # heads per core = 8
CE = HPC * DH         # embed cols per core = 512
P = 128
NQT = S // P          # 16 k-tiles of 128
QC = 4                # q chunks of 512
ET = E // P           # 8 e-tiles
MT = CE // P          # 4 output dim tiles (head pairs)

F32 = mybir.dt.float32
BF16 = mybir.dt.bfloat16
I16 = mybir.dt.int16
AF = mybir.ActivationFunctionType

# Q is pre-scaled by A_SCALE on the host so PSUM scores are the Schraudolph
# exponent: exp(s/8) ~ bf16_frombits(round(A*s + 16248)).
A_SCALE = 16.0 / np.log(2.0)          # 23.0831...
EXP_SCALE = float(np.log(2.0) / 128.0)  # ACT exp scale on pre-scaled scores
SCH_BIAS = 16248.0                      # 127*128 - 8 (centering bias)
_DVE_EXP = os.environ.get("KERNEL_DVE_EXP", "1") == "1"


def _build(tc, out, hs, wq, bq, wk, bk, wv, bv):
    nc = tc.nc
    with ExitStack() as ctx:
        persist = ctx.enter_context(tc.tile_pool(name="persist", bufs=1))
        xtp = ctx.enter_context(tc.tile_pool(name="xt_pool", bufs=1))
        ep = ctx.enter_context(tc.tile_pool(name="e_pool", bufs=12))
        cp = ctx.enter_context(tc.tile_pool(name="c_pool", bufs=4))
        otp = ctx.enter_context(tc.tile_pool(name="ot_pool", bufs=4))
        zp = ctx.enter_context(tc.tile_pool(name="z_pool", bufs=2))
        drp = ctx.enter_context(tc.tile_pool(name="dram_pool", bufs=2, space="DRAM"))
        pjp = ctx.enter_context(tc.tile_pool(name="proj_psum", bufs=2, space="PSUM"))

        # ---- persistent buffers ----
        qt = [persist.tile([P, S], BF16, tag=f"qt{m}", name=f"qt{m}")
              for m in range(MT)]
        kt = [persist.tile([P, S], BF16, tag=f"kt{m}", name=f"kt{m}")
              for m in range(MT)]
        v = [persist.tile([P, HPC, DH + 1], BF16, tag=f"v{st}", name=f"v{st}")
             for st in range(NQT)]
        xtall = xtp.tile([P, ET, S], BF16, tag="xtall", name="xtall")
        xt = [xtall[:, e] for e in range(ET)]
        ws = {}
        for nm in ("wq", "wk", "wv"):
            ws[nm] = persist.tile([P, ET, CE], BF16, tag=nm, name=nm)

        # X arrives host-transposed (xt_dram [E, S] bf16): plain strided DMAs,
        # 16 loads split across the two HWDGE queues (sync + scalar) so the
        # first q-chunks land within ~8us. W quarters ride the gpsimd SWDGE
        # rings in parallel.
        for e in range(ET):
            eng = nc.sync if e % 2 == 0 else nc.scalar
            eng.dma_start(xtall[:, e, :], hs[ts(e, P), :])

        def load_w(nm, wsrc):
            # W arrives host-rearranged to [p, (o c)]: fully contiguous per
            # partition, so each half is few-descriptor and rides the ring
            # at full speed.
            for qq in range(2):
                nc.gpsimd.dma_start(
                    ws[nm][:, ts(qq, 4), :],
                    wsrc[:, ts(qq, 4 * CE)].rearrange(
                        "p (o c) -> p o c", o=4),
                )

        load_w("wq", wq)
        load_w("wk", wk)
        load_w("wv", wv)

        bqs = persist.tile([P, MT], F32, tag="bqs")
        bks = persist.tile([P, MT], F32, tag="bks")
        nc.gpsimd.dma_start(bqs, bq.rearrange("(o p) -> p o", p=P))
        nc.gpsimd.dma_start(bks, bk.rearrange("(o p) -> p o", p=P))
        # bv row padded to a [128, 512] tile (row 0 = bv, rest 0) so the
        # V-bias matmul is K=128 (no PE tiling-mode switch).
        bvp = persist.tile([P, CE], BF16, tag="bvp")
        nc.vector.memset(bvp, 0.0)
        nc.gpsimd.dma_start(bvp[0:1, :], bv[None, :])
        unit_row = persist.tile([P, P], BF16, tag="unit_row")
        nc.vector.memset(unit_row, 0.0)
        nc.vector.memset(unit_row[0:1, :], 1.0)
        ones_col = persist.tile([P, HPC], BF16, tag="ones_col")
        nc.vector.memset(ones_col, 1.0)
        # touch Exp at t=0 so the ~2.7us ACT table load is off the hot path
        warm = persist.tile([1, 8], F32, tag="warm")
        nc.vector.memset(warm, 0.0)
        nc.scalar.activation(warm, warm, AF.Exp)

        def v_proj(st):
            """V projection for s-tile st (+bias via unit-row matmul),
            augmented with a ones column per head."""
            ps = pjp.tile([P, 512], F32, tag="pps", name="pps")
            for e in range(ET):
                nc.tensor.matmul(
                    ps,
                    lhsT=xt[e][:, ts(st, P)],
                    rhs=ws["wv"][:, e, :],
                    start=(e == 0),
                    stop=False,
                )
            nc.tensor.matmul(ps, lhsT=unit_row, rhs=bvp, start=False, stop=True)
            nc.scalar.copy(
                out=v[st][:, :, 0:DH],
                in_=ps.rearrange("p (h d) -> p h d", h=HPC),
            )
            nc.vector.tensor_copy(out=v[st][:, :, DH], in_=ones_col)

        def qk_proj(m, qc):
            """Q and K projections for dim-tile m (head pair m), q-chunk qc."""
            for wname, dstt, bias in (("wq", qt, bqs), ("wk", kt, bks)):
                ps = pjp.tile([P, 512], F32, tag="pps", name="pps")
                for e in range(ET):
                    nc.tensor.matmul(
                        ps,
                        lhsT=ws[wname][:, e, ts(m, P)],
                        rhs=xt[e][:, ts(qc, 512)],
                        start=(e == 0),
                        stop=(e == ET - 1),
                    )
                # bias-add on DVE: ACT carries the critical odd-exp edge,
                # so its queue stays clear of the pair-boundary bias bursts.
                nc.vector.tensor_scalar_add(
                    dstt[m][:, ts(qc, 512)], ps, bias[:, ts(m, 1)],
                )

        deferred = []  # one pending DVE-epilogue closure, run 1 unit later

        def attention_pair(pr, fill=None):
            """Full attention for head pair pr (heads 2pr, 2pr+1).

            K-tiles are processed in pairs (one 64-row-mode scores block,
            one 128-row-mode ctx block per two k-tiles) to halve the PE
            tiling-mode drains. exp alternates ACT (even k-tile) / DVE
            Schraudolph (odd k-tile).

            fill: optional dict {(qc, kt2): [thunk, ...]} of extra work
            emitted between the scores and ctx blocks of that iteration.
            """
            hA, hB = 2 * pr, 2 * pr + 1
            for qc in range(QC):
                ctxA = cpp.tile([DH + 1, 512], F32, tag="ctx", name="ctx")
                ctxB = cpp.tile([DH + 1, 512], F32, tag="ctx", name="ctx")
                ets = {}
                for kt2 in range(0, NQT + 4, 2):
                    if kt2 < NQT:
                        for kk in (kt2, kt2 + 1):
                            sps = spp.tile([P, 1024], F32, tag="sps",
                                           name="sps")
                            nc.tensor.matmul(
                                sps[:, 0:512],
                                lhsT=kt[pr][0:DH, ts(kk, P)],
                                rhs=qt[pr][0:DH, ts(qc, 512)],
                                start=True, stop=True,
                            )
                            nc.tensor.matmul(
                                sps[:, 512:1024],
                                lhsT=kt[pr][DH:P, ts(kk, P)],
                                rhs=qt[pr][DH:P, ts(qc, 512)],
                                start=True, stop=True,
                            )
                            et = ep.tile([P, 1024], BF16, tag="expT",
                                         name="expT")
                            # DVE takes EVEN k-tiles: the scheduler's
                            # binding edge is scores(k+1) <- exp(k-1), so
                            # the odd (critical) exps go to the faster,
                            # less-congested ACT engine.
                            if _DVE_EXP and kk % 2 == 0:
                                nc.vector.tensor_scalar_add(
                                    et.bitcast(I16), sps, SCH_BIAS)
                            else:
                                nc.scalar.activation(et, sps, AF.Exp,
                                                     scale=EXP_SCALE)
                            ets[kk] = et
                    if fill is not None:
                        for thunk in fill.get((qc, kt2), ()):
                            thunk()
                    if kt2 == 4:
                        # previous unit's DVE epilogue: by now its Z bounce
                        # has landed, and the DVE burst no longer delays
                        # this unit's early exp tiles.
                        while deferred:
                            deferred.pop(0)()
                    for pk in (kt2 - 4, kt2 - 3):
                        if 0 <= pk < NQT:
                            pe_ = ets.pop(pk)
                            nc.tensor.matmul(
                                ctxA, lhsT=v[pk][:, hA, :], rhs=pe_[:, 0:512],
                                start=(pk == 0), stop=(pk == NQT - 1),
                            )
                            nc.tensor.matmul(
                                ctxB, lhsT=v[pk][:, hB, :],
                                rhs=pe_[:, 512:1024],
                                start=(pk == 0), stop=(pk == NQT - 1),
                            )

                # normalize + transpose + store via ACT/DVE/DMA (no PE).
                # Order: cs drains + zd writes first, then both block
                # transposes (DVE busy during the zd->c2 DRAM round-trip),
                # then reciprocal + scales + stores.
                zd = drp.tile([2, 2, 512], F32, tag="zd", name="zd")
                css = []
                for idx, ctx_ps in enumerate((ctxA, ctxB)):
                    cs = cp.tile([DH + 1, 512], F32, tag="cs", name="cs")
                    # ctx drain on DVE: keeps the ACT queue clear for the
                    # critical odd-exp edge at unit boundaries.
                    nc.vector.tensor_copy(out=cs, in_=ctx_ps)
                    nc.sync.dma_start(zd[0, idx][None, :], cs[DH : DH + 1, :])
                    css.append(cs)
                c2 = zp.tile([DH, 2, NQT], F32, tag="c2", name="c2")
                for i in range(2):
                    nc.sync.dma_start(
                        c2[ts(i, 32)],
                        zd[0].rearrange("h (j a) -> a h j", a=32),
                    )
                def dve_epilogue(css=css, c2=c2, qc=qc, hA=hA, hB=hB):
                    bts = []
                    for idx in range(2):
                        bt = otp.tile([DH, 512], F32, tag="bt", name="bt")
                        nc.vector.transpose(bt, css[idx][0:DH, :])
                        bts.append(bt)
                    nc.vector.reciprocal(c2, c2)
                    for idx, hl in ((0, hA), (1, hB)):
                        ot = otp.tile([DH, NQT, 32], BF16, tag="ot", name="ot")
                        nc.vector.tensor_tensor(
                            ot,
                            bts[idx].rearrange("p (j b) -> p j b", b=32),
                            c2[:, idx, :, None].to_broadcast([DH, NQT, 32]),
                            mybir.AluOpType.mult,
                        )
                        # store in native block layout (host un-permutes):
                        # out[qc, hl, i, a, j, b] = ctx row qc*512+32j+a,
                        # col hl*64+32i+b. Plain 2KB-contiguous per partition.
                        for i in range(2):
                            eng = nc.sync if i == 0 else nc.scalar
                            eng.dma_start(out[qc, hl, i], ot[ts(i, 32)])

                deferred.append(dve_epilogue)

        spp = ctx.enter_context(tc.tile_pool(name="s_psum", bufs=2, space="PSUM"))
        cpp = ctx.enter_context(tc.tile_pool(name="ctx_psum", bufs=2, space="PSUM"))

        qk_proj(0, 0)

        # pair-0 fills: V chains two k-tiles ahead of their first ctx use;
        # Q/K chunk c projected two iterations before scores need it.
        fill0 = {}

        def add_fill(qc, kt2, thunk):
            fill0.setdefault((qc, kt2), []).append(thunk)

        for kt2 in range(0, NQT, 2):
            add_fill(0, kt2, (lambda z: lambda: v_proj(z))(kt2))
            add_fill(0, kt2, (lambda z: lambda: v_proj(z + 1))(kt2))
        add_fill(0, 2, lambda: qk_proj(0, 1))
        add_fill(0, 6, lambda: qk_proj(0, 2))
        add_fill(0, 10, lambda: qk_proj(0, 3))

        attention_pair(0, fill=fill0)
        for pr in range(1, MT):
            for qc in range(QC):
                qk_proj(pr, qc)
            attention_pair(pr)
        while deferred:
            deferred.pop(0)()


def build_program():
    from concourse import bacc

    nc = bacc.Bacc("TRN2", target_bir_lowering=False, debug=False)
    hs = nc.dram_tensor("hs", [E, S], BF16, kind="ExternalInput").ap()
    wq = nc.dram_tensor("wq", [P, ET * CE], BF16, kind="ExternalInput").ap()
    bq = nc.dram_tensor("bq", [CE], F32, kind="ExternalInput").ap()
    wk = nc.dram_tensor("wk", [P, ET * CE], BF16, kind="ExternalInput").ap()
    bk = nc.dram_tensor("bk", [CE], F32, kind="ExternalInput").ap()
    wv = nc.dram_tensor("wv", [P, ET * CE], BF16, kind="ExternalInput").ap()
    bv = nc.dram_tensor("bv", [CE], BF16, kind="ExternalInput").ap()
    out = nc.dram_tensor("out", [QC, HPC, 2, 32, NQT, 32], BF16,
                         kind="ExternalOutput").ap()
    with tile.TileContext(nc) as tc:
        _build(tc, out, hs, wq, bq, wk, bk, wv, bv)
    nc.compile()
    return nc


def _to_bf16(x):
    import ml_dtypes

    return np.asarray(x, dtype=np.float32).astype(ml_dtypes.bfloat16)


def _w_layout(w):
    """[E, CE] -> on-chip layout [p, (o c)] with row index o*128+p."""
    return np.ascontiguousarray(
        w.reshape(ET, P, CE).transpose(1, 0, 2).reshape(P, ET * CE))


def make_in_maps(inputs):
    """Slice full inputs into 8 per-core input maps (X/W host-cast to bf16,
    Q projection pre-scaled by 16/ln2)."""
    hsf = np.asarray(inputs["hidden_states"], dtype=np.float32)
    w = {k: np.asarray(inputs[k], dtype=np.float32) for k in
         ("Wq", "bq", "Wk", "bk", "Wv", "bv")}
    in_maps = []
    for core in range(NCORES):
        b, hg = core // HG, core % HG
        cols = slice(hg * CE, (hg + 1) * CE)
        in_maps.append({
            "hs": np.ascontiguousarray(_to_bf16(hsf[b]).T),
            "wq": _w_layout(_to_bf16(w["Wq"][:, cols] * A_SCALE)),
            "bq": np.ascontiguousarray(w["bq"][cols] * A_SCALE),
            "wk": _w_layout(_to_bf16(w["Wk"][:, cols])),
            "bk": np.ascontiguousarray(w["bk"][cols]),
            "wv": _w_layout(_to_bf16(w["Wv"][:, cols])),
            "bv": _to_bf16(w["bv"][cols]),
        })
    return in_maps


def assemble(results):
    """Gather 8 per-core [S, CE] outputs into the full [B, S, E] output."""
    full = np.empty((B, S, E), dtype=np.float32)
    for core in range(NCORES):
        b, hg = core // HG, core % HG
        # raw layout [qc, h, i, a, j, b] -> row qc*512+32j+a, col 64h+32i+b
        r = np.asarray(results[core]["out"], dtype=np.float32)
        r = r.transpose(0, 4, 3, 1, 2, 5).reshape(S, CE)
        full[b, :, hg * CE : (hg + 1) * CE] = r
    return full


_NC_CACHE = None


def kernel(**inputs):
    global _NC_CACHE
    from concourse.bass_utils import run_bass_kernel_spmd

    if _NC_CACHE is None:
        _NC_CACHE = build_program()
    res = run_bass_kernel_spmd(_NC_CACHE, make_in_maps(inputs),
                               core_ids=list(range(NCORES)))
    return assemble(res.results)


# revision 56
# speedup vs baseline: 1.0439x; 1.0439x over previous
"""Multi-head attention (B=4, S=2048, E=1024, H=16, D=64) on 8 trn2 cores.

Sharding: core c handles batch b=c//2 and head-group hg=c%2 (8 heads, 512
embed cols). QKV projection weights are column-sharded by head group so
attention is fully local per device.

Per-core plan (bf16 matmul operands, fp32 PSUM accumulation):
  - X and W are cast to bf16 on the host; XT[e,q] is loaded straight from
    DRAM via xbar transpose-DMA (no PE transposes, no staging copies).
  - Wq/bq are pre-scaled by 16/ln2 on the host so the PSUM scores are
    directly the Schraudolph exponent; the ACT exp path compensates with
    scale=ln2/128.
  - QT[d,q], KT[d,q] = W.T @ XT (+bias via ACT Identity); V[s,d] (+bias via
    a K=128 unit-row matmul so the PE never leaves 128x128 mode), stored
    augmented [V | 1] per head.
  - scores^T[k,q] = KT.T @ QT per head-pair: two K=64 matmuls row-tiled at
    partition bases 0/64 (concurrent in the PE array). K-tiles are batched
    in PAIRS so the 64-row <-> 128-row tiling-mode drain is paid once per
    two k-tiles instead of twice per one.
  - e = exp(scores): even k-tiles on ACT (exact exp, scale=ln2/128), odd
    k-tiles on DVE via the Schraudolph bit-trick (round(sps+16248) as int16
    reinterpreted as bf16) to split the exp throughput across two engines.
  - ctxT_aug[65,q] += [V|1].T @ e ; row 64 = Z (softmax denominator).
  - output: DVE 32x32 block-transpose + per-q 1/Z scale (Z columns via a
    tiny DRAM bounce) + block-permuted DMA to DRAM. PSUM->SBUF ctx drains
    on ACT.
  - Prologue: XT transpose-DMAs on the sync queue (q-chunk major), W/bias
    loads on the gpsimd queue in parallel; V and Q/K chunk projections are
    injected just-in-time inside pair 0's first q-chunk.

Baseline 396us/core -> this version targets ~290us.
"""

import os
import numpy as np
from contextlib import ExitStack

import concourse.bass as bass
import concourse.mybir as mybir
import concourse.tile as tile
from concourse.bass import ts, ds

B, S, E = 4, 2048, 1024
H, DH = 16, 64
NCORES = 8
HG = 2                # head groups per batch (cores per batch element)
HPC = H // HG         # heads per core = 8
CE = HPC * DH         # embed cols per core = 512
P = 128
NQT = S // P          # 16 k-tiles of 128
QC = 4                # q chunks of 512
ET = E // P           # 8 e-tiles
MT = CE // P          # 4 output dim tiles (head pairs)

F32 = mybir.dt.float32
BF16 = mybir.dt.bfloat16
I16 = mybir.dt.int16
AF = mybir.ActivationFunctionType

# Q is pre-scaled by A_SCALE on the host so PSUM scores are the Schraudolph
# exponent: exp(s/8) ~ bf16_frombits(round(A*s + 16248)).
A_SCALE = 16.0 / np.log(2.0)          # 23.0831...
EXP_SCALE = float(np.log(2.0) / 128.0)  # ACT exp scale on pre-scaled scores
SCH_BIAS = 16248.0                      # 127*128 - 8 (centering bias)
_DVE_EXP = os.environ.get("KERNEL_DVE_EXP", "1") == "1"


def _build(tc, out, hs, wq, bq, wk, bk, wv, bv):
    nc = tc.nc
    with ExitStack() as ctx:
        persist = ctx.enter_context(tc.tile_pool(name="persist", bufs=1))
        xtp = ctx.enter_context(tc.tile_pool(name="xt_pool", bufs=1))
        ep = ctx.enter_context(tc.tile_pool(name="e_pool", bufs=12))
        cp = ctx.enter_context(tc.tile_pool(name="c_pool", bufs=4))
        otp = ctx.enter_context(tc.tile_pool(name="ot_pool", bufs=4))
        zp = ctx.enter_context(tc.tile_pool(name="z_pool", bufs=2))
        drp = ctx.enter_context(tc.tile_pool(name="dram_pool", bufs=2, space="DRAM"))
        pjp = ctx.enter_context(tc.tile_pool(name="proj_psum", bufs=2, space="PSUM"))

        # ---- persistent buffers ----
        qt = [persist.tile([P, S], BF16, tag=f"qt{m}", name=f"qt{m}")
              for m in range(MT)]
        kt = [persist.tile([P, S], BF16, tag=f"kt{m}", name=f"kt{m}")
              for m in range(MT)]
        v = [persist.tile([P, HPC, DH + 1], BF16, tag=f"v{st}", name=f"v{st}")
             for st in range(NQT)]
        xtall = xtp.tile([P, ET, S], BF16, tag="xtall", name="xtall")
        xt = [xtall[:, e] for e in range(ET)]
        ws = {}
        for nm in ("wq", "wk", "wv"):
            ws[nm] = persist.tile([P, ET, CE], BF16, tag=nm, name=nm)

        # X arrives host-transposed (xt_dram [E, S] bf16): plain strided DMAs,
        # 16 loads split across the two HWDGE queues (sync + scalar) so the
        # first q-chunks land within ~8us. W quarters ride the gpsimd SWDGE
        # rings in parallel.
        for e in range(ET):
            eng = nc.sync if e % 2 == 0 else nc.scalar
            eng.dma_start(xtall[:, e, :], hs[ts(e, P), :])

        def load_w(nm, wsrc):
            # W arrives host-rearranged to [p, (o c)]: fully contiguous per
            # partition, so each half is few-descriptor and rides the ring
            # at full speed.
            for qq in range(2):
                nc.gpsimd.dma_start(
                    ws[nm][:, ts(qq, 4), :],
                    wsrc[:, ts(qq, 4 * CE)].rearrange(
                        "p (o c) -> p o c", o=4),
                )

        load_w("wq", wq)
        load_w("wk", wk)
        load_w("wv", wv)

        bqs = persist.tile([P, MT], F32, tag="bqs")
        bks = persist.tile([P, MT], F32, tag="bks")
        nc.gpsimd.dma_start(bqs, bq.rearrange("(o p) -> p o", p=P))
        nc.gpsimd.dma_start(bks, bk.rearrange("(o p) -> p o", p=P))
        # bv row padded to a [128, 512] tile (row 0 = bv, rest 0) so the
        # V-bias matmul is K=128 (no PE tiling-mode switch).
        bvp = persist.tile([P, CE], BF16, tag="bvp")
        nc.vector.memset(bvp, 0.0)
        nc.gpsimd.dma_start(bvp[0:1, :], bv[None, :])
        unit_row = persist.tile([P, P], BF16, tag="unit_row")
        nc.vector.memset(unit_row, 0.0)
        nc.vector.memset(unit_row[0:1, :], 1.0)
        ones_col = persist.tile([P, HPC], BF16, tag="ones_col")
        nc.vector.memset(ones_col, 1.0)
        # touch Exp at t=0 so the ~2.7us ACT table load is off the hot path
        warm = persist.tile([1, 8], F32, tag="warm")
        nc.vector.memset(warm, 0.0)
        nc.scalar.activation(warm, warm, AF.Exp)

        def v_proj(st):
            """V projection for s-tile st (+bias via unit-row matmul),
            augmented with a ones column per head."""
            ps = pjp.tile([P, 512], F32, tag="pps", name="pps")
            for e in range(ET):
                nc.tensor.matmul(
                    ps,
                    lhsT=xt[e][:, ts(st, P)],
                    rhs=ws["wv"][:, e, :],
                    start=(e == 0),
                    stop=False,
                )
            nc.tensor.matmul(ps, lhsT=unit_row, rhs=bvp, start=False, stop=True)
            nc.scalar.copy(
                out=v[st][:, :, 0:DH],
                in_=ps.rearrange("p (h d) -> p h d", h=HPC),
            )
            nc.vector.tensor_copy(out=v[st][:, :, DH], in_=ones_col)

        def qk_proj(m, qc):
            """Q and K projections for dim-tile m (head pair m), q-chunk qc."""
            for wname, dstt, bias in (("wq", qt, bqs), ("wk", kt, bks)):
                ps = pjp.tile([P, 512], F32, tag="pps", name="pps")
                for e in range(ET):
                    nc.tensor.matmul(
                        ps,
                        lhsT=ws[wname][:, e, ts(m, P)],
                        rhs=xt[e][:, ts(qc, 512)],
                        start=(e == 0),
                        stop=(e == ET - 1),
                    )
                # bias-add on DVE: ACT carries the critical odd-exp edge,
                # so its queue stays clear of the pair-boundary bias bursts.
                nc.vector.tensor_scalar_add(
                    dstt[m][:, ts(qc, 512)], ps, bias[:, ts(m, 1)],
                )

        deferred = []  # one pending DVE-epilogue closure, run 1 unit later

        def attention_pair(pr, fill=None):
            """Full attention for head pair pr (heads 2pr, 2pr+1).

            K-tiles are processed in pairs (one 64-row-mode scores block,
            one 128-row-mode ctx block per two k-tiles) to halve the PE
            tiling-mode drains. exp alternates ACT (even k-tile) / DVE
            Schraudolph (odd k-tile).

            fill: optional dict {(qc, kt2): [thunk, ...]} of extra work
            emitted between the scores and ctx blocks of that iteration.
            """
            hA, hB = 2 * pr, 2 * pr + 1
            for qc in range(QC):
                ctxA = cpp.tile([DH + 1, 512], F32, tag="ctx", name="ctx")
                ctxB = cpp.tile([DH + 1, 512], F32, tag="ctx", name="ctx")
                ets = {}
                for kt2 in range(0, NQT + 4, 2):
                    if kt2 < NQT:
                        for kk in (kt2, kt2 + 1):
                            sps = spp.tile([P, 1024], F32, tag="sps",
                                           name="sps")
                            nc.tensor.matmul(
                                sps[:, 0:512],
                                lhsT=kt[pr][0:DH, ts(kk, P)],
                                rhs=qt[pr][0:DH, ts(qc, 512)],
                                start=True, stop=True,
                            )
                            nc.tensor.matmul(
                                sps[:, 512:1024],
                                lhsT=kt[pr][DH:P, ts(kk, P)],
                                rhs=qt[pr][DH:P, ts(qc, 512)],
                                start=True, stop=True,
                            )
                            et = ep.tile([P, 1024], BF16, tag="expT",
                                         name="expT")
                            # DVE takes EVEN k-tiles: the scheduler's
                            # binding edge is scores(k+1) <- exp(k-1), so
                            # the odd (critical) exps go to the faster,
                            # less-congested ACT engine.
                            if _DVE_EXP and kk % 2 == 0:
                                nc.vector.tensor_scalar_add(
                                    et.bitcast(I16), sps, SCH_BIAS)
                            else:
                                nc.scalar.activation(et, sps, AF.Exp,
                                                     scale=EXP_SCALE)
                            ets[kk] = et
                    if fill is not None:
                        for thunk in fill.get((qc, kt2), ()):
                            thunk()
                    if kt2 == 4:
                        # previous unit's DVE epilogue: by now its Z bounce
                        # has landed, and the DVE burst no longer delays
                        # this unit's early exp tiles.
                        while deferred:
                            deferred.pop(0)()
                    for pk in (kt2 - 4, kt2 - 3):
                        if 0 <= pk < NQT:
                            pe_ = ets.pop(pk)
                            nc.tensor.matmul(
                                ctxA, lhsT=v[pk][:, hA, :], rhs=pe_[:, 0:512],
                                start=(pk == 0), stop=(pk == NQT - 1),
                            )
                            nc.tensor.matmul(
                                ctxB, lhsT=v[pk][:, hB, :],
                                rhs=pe_[:, 512:1024],
                                start=(pk == 0), stop=(pk == NQT - 1),
                            )

                # normalize + transpose + store via ACT/DVE/DMA (no PE).
                # Order: cs drains + zd writes first, then both block
                # transposes (DVE busy during the zd->c2 DRAM round-trip),
                # then reciprocal + scales + stores.
                zd = drp.tile([2, 2, 512], F32, tag="zd", name="zd")
                css = []
                for idx, ctx_ps in enumerate((ctxA, ctxB)):
                    cs = cp.tile([DH + 1, 512], F32, tag="cs", name="cs")
                    nc.scalar.copy(cs, ctx_ps)
                    nc.sync.dma_start(zd[0, idx][None, :], cs[DH : DH + 1, :])
                    css.append(cs)
                c2 = zp.tile([DH, 2, NQT], F32, tag="c2", name="c2")
                for i in range(2):
                    nc.sync.dma_start(
                        c2[ts(i, 32)],
                        zd[0].rearrange("h (j a) -> a h j", a=32),
                    )
                def dve_epilogue(css=css, c2=c2, qc=qc, hA=hA, hB=hB):
                    bts = []
                    for idx in range(2):
                        bt = otp.tile([DH, 512], F32, tag="bt", name="bt")
                        nc.vector.transpose(bt, css[idx][0:DH, :])
                        bts.append(bt)
                    nc.vector.reciprocal(c2, c2)
                    for idx, hl in ((0, hA), (1, hB)):
                        ot = otp.tile([DH, NQT, 32], BF16, tag="ot", name="ot")
                        nc.vector.tensor_tensor(
                            ot,
                            bts[idx].rearrange("p (j b) -> p j b", b=32),
                            c2[:, idx, :, None].to_broadcast([DH, NQT, 32]),
                            mybir.AluOpType.mult,
                        )
                        # store in native block layout (host un-permutes):
                        # out[qc, hl, i, a, j, b] = ctx row qc*512+32j+a,
                        # col hl*64+32i+b. Plain 2KB-contiguous per partition.
                        for i in range(2):
                            eng = nc.sync if i == 0 else nc.scalar
                            eng.dma_start(out[qc, hl, i], ot[ts(i, 32)])

                deferred.append(dve_epilogue)

        spp = ctx.enter_context(tc.tile_pool(name="s_psum", bufs=2, space="PSUM"))
        cpp = ctx.enter_context(tc.tile_pool(name="ctx_psum", bufs=2, space="PSUM"))

        qk_proj(0, 0)

        # pair-0 fills: V chains two k-tiles ahead of their first ctx use;
        # Q/K chunk c projected two iterations before scores need it.
        fill0 = {}

        def add_fill(qc, kt2, thunk):
            fill0.setdefault((qc, kt2), []).append(thunk)

        for kt2 in range(0, NQT, 2):
            add_fill(0, kt2, (lambda z: lambda: v_proj(z))(kt2))
            add_fill(0, kt2, (lambda z: lambda: v_proj(z + 1))(kt2))
        add_fill(0, 2, lambda: qk_proj(0, 1))
        add_fill(0, 6, lambda: qk_proj(0, 2))
        add_fill(0, 10, lambda: qk_proj(0, 3))

        attention_pair(0, fill=fill0)
        for pr in range(1, MT):
            for qc in range(QC):
                qk_proj(pr, qc)
            attention_pair(pr)
        while deferred:
            deferred.pop(0)()


def build_program():
    from concourse import bacc

    nc = bacc.Bacc("TRN2", target_bir_lowering=False, debug=False)
    hs = nc.dram_tensor("hs", [E, S], BF16, kind="ExternalInput").ap()
    wq = nc.dram_tensor("wq", [P, ET * CE], BF16, kind="ExternalInput").ap()
    bq = nc.dram_tensor("bq", [CE], F32, kind="ExternalInput").ap()
    wk = nc.dram_tensor("wk", [P, ET * CE], BF16, kind="ExternalInput").ap()
    bk = nc.dram_tensor("bk", [CE], F32, kind="ExternalInput").ap()
    wv = nc.dram_tensor("wv", [P, ET * CE], BF16, kind="ExternalInput").ap()
    bv = nc.dram_tensor("bv", [CE], BF16, kind="ExternalInput").ap()
    out = nc.dram_tensor("out", [QC, HPC, 2, 32, NQT, 32], BF16,
                         kind="ExternalOutput").ap()
    with tile.TileContext(nc) as tc:
        _build(tc, out, hs, wq, bq, wk, bk, wv, bv)
    nc.compile()
    return nc


def _to_bf16(x):
    import ml_dtypes

    return np.asarray(x, dtype=np.float32).astype(ml_dtypes.bfloat16)


def _w_layout(w):
    """[E, CE] -> on-chip layout [p, (o c)] with row index o*128+p."""
    return np.ascontiguousarray(
        w.reshape(ET, P, CE).transpose(1, 0, 2).reshape(P, ET * CE))


def make_in_maps(inputs):
    """Slice full inputs into 8 per-core input maps (X/W host-cast to bf16,
    Q projection pre-scaled by 16/ln2)."""
    hsf = np.asarray(inputs["hidden_states"], dtype=np.float32)
    w = {k: np.asarray(inputs[k], dtype=np.float32) for k in
         ("Wq", "bq", "Wk", "bk", "Wv", "bv")}
    in_maps = []
    for core in range(NCORES):
        b, hg = core // HG, core % HG
        cols = slice(hg * CE, (hg + 1) * CE)
        in_maps.append({
            "hs": np.ascontiguousarray(_to_bf16(hsf[b]).T),
            "wq": _w_layout(_to_bf16(w["Wq"][:, cols] * A_SCALE)),
            "bq": np.ascontiguousarray(w["bq"][cols] * A_SCALE),
            "wk": _w_layout(_to_bf16(w["Wk"][:, cols])),
            "bk": np.ascontiguousarray(w["bk"][cols]),
            "wv": _w_layout(_to_bf16(w["Wv"][:, cols])),
            "bv": _to_bf16(w["bv"][cols]),
        })
    return in_maps


def assemble(results):
    """Gather 8 per-core [S, CE] outputs into the full [B, S, E] output."""
    full = np.empty((B, S, E), dtype=np.float32)
    for core in range(NCORES):
        b, hg = core // HG, core % HG
        # raw layout [qc, h, i, a, j, b] -> row qc*512+32j+a, col 64h+32i+b
        r = np.asarray(results[core]["out"], dtype=np.float32)
        r = r.transpose(0, 4, 3, 1, 2, 5).reshape(S, CE)
        full[b, :, hg * CE : (hg + 1) * CE] = r
    return full


_NC_CACHE = None


def kernel(**inputs):
    global _NC_CACHE
    from concourse.bass_utils import run_bass_kernel_spmd

    if _NC_CACHE is None:
        _NC_CACHE = build_program()
    res = run_bass_kernel_spmd(_NC_CACHE, make_in_maps(inputs),
                               core_ids=list(range(NCORES)))
    return assemble(res.results)
